# revision 4
# baseline (speedup 1.0000x reference)
"""AttentionBlock (GroupNorm -> qkv -> attention -> proj -> residual) on 8 TRN2 cores.

Data-parallel over batch: B=8 samples, one per NeuronCore; no collectives.

All matmuls run in fp8e4 (e4m3) with DoubleRow perf mode: two 128-deep
K-subtiles contracted per matmul at double rate. Layouts put K-subtile
pairs adjacent in the free dim so a single 3D AP [128, 2, N] feeds each
DoubleRow operand (dual-fp8 ldweights requires a 128-wide stationary
free dim — ones vectors must be [128, 2, 128], which conveniently makes
the softmax-denominator matmul produce its result replicated across all
128 PSUM partitions, i.e. pre-broadcast):

  - h, q, k stored [128, NCT=4, T]   (partition = channel%128, dim1 = c-subtile)
  - vT stored     [128, NT=32, C]    (partition = token%128,  dim1 = s-subtile)
  - PT (exp of scores^T) [128, NT, TCH] per t-chunk, fp8
  - weights wqkvT [128, NCT, 3C], wprojT [128, NCT, C], fp8

Scores are computed transposed, ST[s,t] = sum_c k[c,s] q[c,t], so softmax's
reduction lands on partitions: exp on the ACT engine (with a constant -3
offset so all exps fit fp8 range; the offset cancels in normalization),
denominator via the DoubleRow ones-matmul above, one DVE reciprocal of the
replicated denominator gives the broadcast 1/den directly.

Schedule: the ACT engine (256 exp tiles) is the phase-3 bottleneck, so
everything else is arranged to keep its stream gapless: den+PV consume
exp'd pairs two pairs late (ACT latency off the PE in-order path), and
each chunk's last pair-consumes + normalization + proj run inside the NEXT
chunk's scores window (software pipeline across the chunk boundary).
PSUM budget (8 banks): scores 2 + den 1 + PV 4 + proj 1.

GroupNorm: bn_stats on chunked x loads (x DMAd via the SP/ACT HWDGE
queues, SBUF-resident f32 for the residual add), cross-partition group
reduce and broadcast-back via tiny exact 0/1-selection f32 matmuls.
GroupNorm's apply pass (fp8 h) is fused into the qkv chunk loop, which
interleaves q/k (ACT evac, +bias) and vT (DVE evac) production.
Weight transposes / q,k pre-scaling by C**-0.25 / v-bias folding are done
on the host in numpy - O(C^2) one-time prep.
"""

import os
import sys

for _p in ("/opt/trn_rl_repo", "/opt/pypackages"):
    if os.path.isdir(_p) and _p not in sys.path:
        sys.path.insert(0, _p)

import numpy as np
import ml_dtypes

import json as _json

import concourse.bass as bass
import concourse.tile as tile
from concourse import mybir
from concourse.bass_utils import run_bass_kernel_spmd

# Walrus's codegen (setupSyncWait) encodes at most ONE sync wait on a DMA
# instruction and errors out ("Too many sync wait commands") instead of
# splitting. Tile's scheduler freely attaches several waits. This pass hoists
# excess waits into standalone EventSemaphore instructions on the same engine
# immediately before the offending instruction — semantically identical (the
# engine's sequencer evaluates them in stream order before issuing it).
_WAIT_LIMITS = {"DMACopy": 1}
_WAIT_LIMIT_DEFAULT = 1


def _legalize_sync_waits(raw: bytes) -> bytes:
    d = _json.loads(raw)
    n_hoisted = 0
    for fn in d.get("functions", []):
        for blk in fn.get("blocks", []):
            out = []
            for inst in blk["instructions"]:
                si = inst.get("sync_info")
                waits = (si or {}).get("on_wait") or []
                limit = _WAIT_LIMITS.get(inst.get("opcode"), _WAIT_LIMIT_DEFAULT)
                if len(waits) > limit and inst.get("engine") not in (
                        None, "Unassigned"):
                    keep = waits[-limit:]
                    hoist = waits[:-limit]
                    for j, w in enumerate(hoist):
                        out.append({
                            "debug": inst.get("debug", 0),
                            "engine": inst["engine"],
                            "ins": [], "outs": [],
                            "name": f"{inst['name']}-hw{j}",
                            "opcode": "EventSemaphore",
                            "sync_info": {"on_update": [], "on_wait": [w]},
                        })
                        n_hoisted += 1
                    si["on_wait"] = keep
                out.append(inst)
            blk["instructions"] = out
    if n_hoisted:
        d.setdefault("attributes", {})
    return _json.dumps(d).encode()


def _install_wait_legalizer(nc):
    orig = nc.to_json_bytes

    def patched():
        return _legalize_sync_waits(orig())

    nc.to_json_bytes = patched

F32 = mybir.dt.float32
BF16 = mybir.dt.bfloat16
F8 = mybir.dt.float8e4
AL = mybir.AluOpType
AF = mybir.ActivationFunctionType
DR = mybir.MatmulPerfMode.DoubleRow

C = 512
G = 32          # groupnorm groups
NCT = C // 128  # 4 channel subtiles
EPS = 1e-5
TCH = 512       # t-chunk width
EXP_OFF = 3.0   # exp(s - OFF): keeps all exps < e^3 ~ 20, inside fp8 range


def build_graph(T, n_cores=8, phases=3):
    NT = T // 128
    NCH = T // TCH
    NST = TCH // 128  # s-subtiles per chunk in phase 2 (v production)
    nc = bass.Bass("TRN2", target_bir_lowering=False, debug=False,
                   num_devices=n_cores)

    x_d = nc.dram_tensor("x", [C, T], F32, kind="ExternalInput").ap()
    wqkvt_d = nc.dram_tensor("wqkvt", [128, NCT, 3 * C], F8,
                             kind="ExternalInput").ap()
    wprojt_d = nc.dram_tensor("wprojt", [128, NCT, C], F8,
                              kind="ExternalInput").ap()
    bqk_d = nc.dram_tensor("bqk", [2 * C, 1], F32, kind="ExternalInput").ap()
    bout_d = nc.dram_tensor("bout", [C, 1], F32, kind="ExternalInput").ap()
    gnw_d = nc.dram_tensor("gnw", [C, 1], F32, kind="ExternalInput").ap()
    gnb_d = nc.dram_tensor("gnb", [C, 1], F32, kind="ExternalInput").ap()
    # 0/1 selection matrices for the cross-partition GroupNorm reductions
    # (replace DRAM round trips with tiny exact f32 matmuls)
    gsel_d = nc.dram_tensor("gsel", [128, NCT, G], F32,
                            kind="ExternalInput").ap()
    gselt_d = nc.dram_tensor("gselt", [G, NCT, 128], F32,
                             kind="ExternalInput").ap()
    out_d = nc.dram_tensor("out", [C, T], F32, kind="ExternalOutput").ap()

    with tile.TileContext(nc) as tc:
        with (
            tc.tile_pool(name="singles", bufs=1) as sing,
            tc.tile_pool(name="persist", bufs=1) as pers,
        ):
            # ---- x loads FIRST, on the HWDGE queues (SP/ACT): everything
            # in phase 1 waits on these, and the Pool SWDGE queue is busy
            # with descriptor-gen for the small weight loads below ----
            x_sb = [pers.tile([128, T], F32, name=f"x{i}", tag=f"x{i}")
                    for i in range(NCT)]
            nxc = 4  # x DMA chunks per channel tile
            xq = [nc.sync, nc.scalar, nc.sync, nc.scalar]
            for ci in range(NCT):
                for j in range(nxc):
                    w = T // nxc
                    xq[ci].dma_start(
                        x_sb[ci][:, j * w:(j + 1) * w],
                        x_d[ci * 128:(ci + 1) * 128, j * w:(j + 1) * w])

            # ---- weights & constants (resident whole kernel) ----
            wqkv_sb = sing.tile([128, NCT, 3 * C], F8, name="wqkv", tag="wqkv")
            nc.gpsimd.dma_start(wqkv_sb, wqkvt_d)
            wproj_sb = sing.tile([128, NCT, C], F8, name="wproj", tag="wproj")
            nc.gpsimd.dma_start(wproj_sb, wprojt_d)
            bq_sb, bk_sb, bout_sb = [], [], []
            for i in range(NCT):
                b = sing.tile([128, 1], F32, name=f"bq{i}", tag=f"bq{i}")
                nc.gpsimd.dma_start(b, bqk_d[i * 128:(i + 1) * 128, :])
                bq_sb.append(b)
            for i in range(NCT):
                b = sing.tile([128, 1], F32, name=f"bk{i}", tag=f"bk{i}")
                nc.gpsimd.dma_start(b, bqk_d[C + i * 128:C + (i + 1) * 128, :])
                bk_sb.append(b)
            for i in range(NCT):
                b = sing.tile([128, 1], F32, name=f"bout{i}", tag=f"bout{i}")
                nc.gpsimd.dma_start(b, bout_d[i * 128:(i + 1) * 128, :])
                bout_sb.append(b)
            # dual-fp8 ldweights requires stationary free dim 128: use a
            # [128, 2, 128] ones stationary so the den matmul lands the
            # denominator REPLICATED across all 128 PSUM partitions (the
            # broadcast comes for free)
            ones8 = sing.tile([128, 2, 128], F8, name="ones8", tag="ones8")
            nc.vector.memset(ones8, 1.0)
            negoff = sing.tile([128, 1], F32, name="negoff", tag="negoff")
            nc.vector.memset(negoff, -EXP_OFF)
            # groupnorm per-channel affine coefs (computed in phase 1)
            A_sb = [sing.tile([128, 1], F32, name=f"gnA{i}", tag=f"gnA{i}")
                    for i in range(NCT)]
            B_sb = [sing.tile([128, 1], F32, name=f"gnB{i}", tag=f"gnB{i}")
                    for i in range(NCT)]

            # ---- persistent activations ----
            h_sb = pers.tile([128, NCT, T], F8, name="h", tag="h")
            q_sb = pers.tile([128, NCT, T], F8, name="q", tag="q")
            k_sb = pers.tile([128, NCT, T], F8, name="k", tag="k")
            vT_sb = pers.tile([128, NT, C], F8, name="vt", tag="vt")

            # ================= phase 1: GroupNorm stats =================
            # Cross-partition group reductions/broadcasts via tiny exact
            # 0/1-selection f32 matmuls (no DRAM round trips). x DMAs are
            # chunked so bn_stats pipelines with the loads.
            with (
                tc.tile_pool(name="gns", bufs=2) as gns,
                tc.tile_pool(name="pgn", bufs=2, space="PSUM") as pgn,
            ):
                gsel_sb = gns.tile([128, NCT, G], F32, name="gsel", tag="gsel",
                                   bufs=1)
                nc.gpsimd.dma_start(gsel_sb, gsel_d)
                gselt_sb = gns.tile([G, NCT, 128], F32, name="gselt",
                                    tag="gselt", bufs=1)
                nc.gpsimd.dma_start(gselt_sb, gselt_d)
                gnw_sb = gns.tile([128, NCT], F32, name="gnw_sb", tag="gnw",
                                  bufs=1)
                gnb_sb = gns.tile([128, NCT], F32, name="gnb_sb", tag="gnb",
                                  bufs=1)
                for ci in range(NCT):
                    nc.gpsimd.dma_start(gnw_sb[:, ci:ci + 1],
                                        gnw_d[ci * 128:(ci + 1) * 128, :])
                    nc.gpsimd.dma_start(gnb_sb[:, ci:ci + 1],
                                        gnb_d[ci * 128:(ci + 1) * 128, :])
                nbn = T // 512
                gst = pgn.tile([G, 2], F32, name="gst_ps", tag="gst")
                mvs = []
                for ci in range(NCT):
                    bns = gns.tile([128, nbn, 6], F32, name="bns", tag="bns")
                    x3 = x_sb[ci].rearrange("p (n f) -> p n f", f=512)
                    for j in range(nbn):
                        nc.vector.bn_stats(bns[:, j, :], x3[:, j, :])
                    mv = gns.tile([128, 2], F32, name="mv", tag=f"mv{ci}",
                                  bufs=1)
                    nc.vector.bn_aggr(mv, bns)
                    # mv[:,1] <- E[x^2] = mu^2 + var
                    nc.vector.scalar_tensor_tensor(
                        mv[:, 1:2], in0=mv[:, 0:1], scalar=mv[:, 0:1],
                        in1=mv[:, 1:2], op0=AL.mult, op1=AL.add)
                    mvs.append(mv)
                # group-sum the per-channel (mean, E[x^2]) rows: [32, 2]
                for ci in range(NCT):
                    nc.tensor.matmul(gst, lhsT=gsel_sb[:, ci, :], rhs=mvs[ci],
                                     start=(ci == 0), stop=(ci == NCT - 1))
                gv = gns.tile([G, 2], F32, name="gv", tag="gv")
                gsize = C // G
                nc.vector.tensor_scalar_mul(gv, gst, 1.0 / gsize)
                std = gns.tile([G, 1], F32, name="std", tag="std")
                # mu^2 - E[x^2] = -var
                nc.vector.scalar_tensor_tensor(
                    std, in0=gv[:, 0:1], scalar=gv[:, 0:1], in1=gv[:, 1:2],
                    op0=AL.mult, op1=AL.subtract)
                # var + eps
                nc.vector.tensor_scalar(std, std, -1.0, EPS,
                                        op0=AL.mult, op1=AL.add)
                nc.scalar.activation(std, std, AF.Sqrt)
                rhsb = gns.tile([G, 2], F32, name="rhsb", tag="rhsb")
                nc.vector.reciprocal(rhsb[:, 0:1], std)
                nc.vector.tensor_copy(rhsb[:, 1:2], gv[:, 0:1])
                # broadcast (rstd, mu) back to the 128-partition channel
                # layout per subtile, then A = gn_w * rstd; B = gn_b - mu * A
                for ci in range(NCT):
                    bc = pgn.tile([128, 2], F32, name="bc_ps", tag="bc")
                    nc.tensor.matmul(bc, lhsT=gselt_sb[:, ci, :], rhs=rhsb,
                                     start=True, stop=True)
                    nc.vector.tensor_mul(A_sb[ci], gnw_sb[:, ci:ci + 1],
                                         bc[:, 0:1])
                    tmp = gns.tile([128, 1], F32, name="gn_tmp", tag="tmp")
                    nc.vector.tensor_mul(tmp, bc[:, 1:2], A_sb[ci])
                    nc.vector.tensor_sub(B_sb[ci], gnb_sb[:, ci:ci + 1], tmp)

            # ========== phase 2: GN apply + qkv, fused per t-chunk ==========
            with tc.tile_pool(name="qkvp", bufs=6, space="PSUM") as qkvp:
                for ch in range(NCH if phases >= 2 else 0):
                    t0 = ch * TCH
                    # h = A*x + B  (f32 -> fp8), per channel subtile
                    for ci in range(NCT):
                        nc.vector.tensor_scalar(
                            h_sb[:, ci, t0:t0 + TCH],
                            x_sb[ci][:, t0:t0 + TCH],
                            A_sb[ci], B_sb[ci], op0=AL.mult, op1=AL.add)

                    def qk_one(dst, dst_off, bias, ci):
                        ps = qkvp.tile([128, TCH], F32, name="qkv_ps",
                                       tag="ps")
                        for p in (0, 2):
                            nc.tensor.matmul(
                                ps,
                                lhsT=wqkv_sb[:, p:p + 2,
                                             dst_off + ci * 128:
                                             dst_off + (ci + 1) * 128],
                                rhs=h_sb[:, p:p + 2, t0:t0 + TCH],
                                start=(p == 0), stop=(p == 2),
                                perf_mode=DR)
                        nc.scalar.activation(
                            dst[:, ci, t0:t0 + TCH], ps,
                            AF.Identity, bias=bias[ci])

                    def v_one(j):
                        st = ch * NST + j
                        ps = qkvp.tile([128, C], F32, name="qkv_ps2",
                                       tag="ps")
                        for p in (0, 2):
                            nc.tensor.matmul(
                                ps,
                                lhsT=h_sb[:, p:p + 2,
                                          st * 128:(st + 1) * 128],
                                rhs=wqkv_sb[:, p:p + 2, 2 * C:3 * C],
                                start=(p == 0), stop=(p == 2),
                                perf_mode=DR)
                        nc.vector.tensor_copy(vT_sb[:, st, :], ps)

                    # interleave so ACT (q,k evacs) and DVE (v evacs) are
                    # both fed continuously
                    for ci in range(NCT):
                        qk_one(q_sb, 0, bq_sb, ci)
                        qk_one(k_sb, C, bk_sb, ci)
                        v_one(ci)

            # ================= phase 3: attention + proj =================
            with (
                tc.tile_pool(name="p3s", bufs=2) as p3s,
                tc.tile_pool(name="p3w", bufs=2) as p3w,
                tc.tile_pool(name="psp", bufs=2, space="PSUM") as psp,
                tc.tile_pool(name="pden", bufs=1, space="PSUM") as pden,
                tc.tile_pool(name="ppv", bufs=1, space="PSUM") as ppv,
                tc.tile_pool(name="ppr", bufs=1, space="PSUM") as ppr,
            ):
                def emit_proj(t0, at):
                    # proj + bias + residual (software-pipelined one chunk
                    # behind the scores/PV of the next chunk)
                    for oi in range(NCT):
                        pr = ppr.tile([128, TCH], F32, name="pr_ps", tag="pr")
                        for p in (0, 2):
                            nc.tensor.matmul(
                                pr,
                                lhsT=wproj_sb[:, p:p + 2,
                                              oi * 128:(oi + 1) * 128],
                                rhs=at[:, p:p + 2, :],
                                start=(p == 0), stop=(p == 2),
                                perf_mode=DR)
                        osb = p3w.tile([128, TCH], F32, name="osb", tag="osb",
                                       bufs=3)
                        nc.vector.scalar_tensor_tensor(
                            osb, in0=pr, scalar=bout_sb[oi],
                            in1=x_sb[oi][:, t0:t0 + TCH],
                            op0=AL.add, op1=AL.add)
                        nc.sync.dma_start(
                            out_d[oi * 128:(oi + 1) * 128, t0:t0 + TCH], osb)

                # Software pipeline: each chunk's last two pair-consumes,
                # normalization, and proj run inside the NEXT chunk's
                # scores window so neither the PE nor the ACT exp stream
                # ever pauses at a chunk boundary.
                def make_chunk_ops(t0, pt, dps, pvs):
                    state = {}

                    def consume_pair(i):
                        nc.tensor.matmul(
                            dps, lhsT=ones8,
                            rhs=pt[:, 2 * i:2 * i + 2, :],
                            start=(i == 0), stop=(i == NT // 2 - 1),
                            perf_mode=DR)
                        for ci in range(NCT):
                            nc.tensor.matmul(
                                pvs[ci],
                                lhsT=vT_sb[:, 2 * i:2 * i + 2,
                                           ci * 128:(ci + 1) * 128],
                                rhs=pt[:, 2 * i:2 * i + 2, :],
                                start=(i == 0), stop=(i == NT // 2 - 1),
                                perf_mode=DR)

                    def finalize():
                        # dps holds den replicated on every partition: one
                        # DVE reciprocal gives the broadcast 1/den in SBUF
                        bcs = p3w.tile([128, TCH], F32, name="bc_sb",
                                       tag="bc_sb")
                        nc.vector.reciprocal(bcs, dps)
                        at = p3w.tile([128, NCT, TCH], F8, name="at",
                                      tag="at")
                        for ci in range(NCT):
                            nc.vector.tensor_mul(at[:, ci, :], pvs[ci], bcs)
                        state["at"] = at

                    def proj():
                        emit_proj(t0, state["at"])

                    return consume_pair, finalize, proj

                prev = None  # (consume_pair, finalize, proj) awaiting tail
                for tci in range(NCH if phases >= 3 else 0):
                    t0 = tci * TCH
                    # scores^T + exp -> PT tiles [s, t] fp8; den + PV
                    # accumulate interleaved, two exp'd pairs behind, so
                    # ACT exp latency stays off the PE critical path
                    pt = p3s.tile([128, NT, TCH], F8, name="pt", tag="pt")
                    dps = pden.tile([128, TCH], F32, name="den_ps", tag="den")
                    pvs = [ppv.tile([128, TCH], F32, name=f"pv{ci}",
                                    tag=f"pv{ci}") for ci in range(NCT)]
                    ops = make_chunk_ops(t0, pt, dps, pvs)
                    consume_pair = ops[0]
                    for st in range(NT):
                        sp = psp.tile([128, TCH], F32, name="st_ps", tag="st")
                        for p in (0, 2):
                            nc.tensor.matmul(
                                sp,
                                lhsT=k_sb[:, p:p + 2,
                                          st * 128:(st + 1) * 128],
                                rhs=q_sb[:, p:p + 2, t0:t0 + TCH],
                                start=(p == 0), stop=(p == 2),
                                perf_mode=DR)
                        nc.scalar.activation(pt[:, st, :], sp, AF.Exp,
                                             bias=negoff)
                        if st % 2 == 1 and st >= 5:
                            consume_pair(st // 2 - 2)
                        if prev is not None:
                            if st == 1:
                                prev[0](NT // 2 - 2)
                            elif st == 3:
                                prev[0](NT // 2 - 1)
                                prev[1]()  # recip + at-mults
                            elif st == 5:
                                prev[2]()  # proj + epilogue
                                prev = None
                    prev = ops
                if prev is not None:
                    prev[0](NT // 2 - 2)
                    prev[0](NT // 2 - 1)
                    prev[1]()
                    prev[2]()
    _install_wait_legalizer(nc)
    return nc


def host_prep(gn_w, gn_b, qkv_w, qkv_b, proj_w, proj_b):
    """One-time O(C^2) weight prep in numpy -> per-core replicated inputs."""
    scale = float(C) ** -0.25
    e4 = ml_dtypes.float8_e4m3
    wq, wk, wv = qkv_w[:C], qkv_w[C:2 * C], qkv_w[2 * C:]
    wt = np.concatenate([wq.T * scale, wk.T * scale, wv.T], axis=1)  # [C, 3C]
    wqkvt = np.ascontiguousarray(
        wt.reshape(NCT, 128, 3 * C).transpose(1, 0, 2)).astype(e4)
    wpt = np.ascontiguousarray(proj_w.T)  # [C, C]
    wprojt = np.ascontiguousarray(
        wpt.reshape(NCT, 128, C).transpose(1, 0, 2)).astype(e4)
    bqk = (qkv_b[:2 * C] * scale).astype(np.float32).reshape(2 * C, 1)
    bout = (proj_w @ qkv_b[2 * C:] + proj_b).astype(np.float32).reshape(C, 1)
    # selection matrices: channel c = ci*128 + p belongs to group c//16
    gsize = C // G
    gsel = np.zeros((128, NCT, G), np.float32)
    gselt = np.zeros((G, NCT, 128), np.float32)
    for ci in range(NCT):
        for p in range(128):
            g = (ci * 128 + p) // gsize
            gsel[p, ci, g] = 1.0
            gselt[g, ci, p] = 1.0
    return {
        "wqkvt": wqkvt, "wprojt": wprojt, "bqk": bqk, "bout": bout,
        "gnw": gn_w.astype(np.float32).reshape(C, 1),
        "gnb": gn_b.astype(np.float32).reshape(C, 1),
        "gsel": gsel, "gselt": gselt,
    }


_graph_cache = {}


def get_graph(T, B):
    key = (T, B)
    if key not in _graph_cache:
        _graph_cache[key] = build_graph(T, n_cores=B)
    return _graph_cache[key]


def run(x, gn_w, gn_b, qkv_w, qkv_b, proj_w, proj_b, trace=False):
    x = np.asarray(x, np.float32)
    B, Cv, H, W = x.shape
    T = H * W
    shared = host_prep(np.asarray(gn_w), np.asarray(gn_b),
                       np.asarray(qkv_w), np.asarray(qkv_b),
                       np.asarray(proj_w), np.asarray(proj_b))
    nc = get_graph(T, B)
    in_maps = []
    for i in range(B):
        m = dict(shared)
        m["x"] = np.ascontiguousarray(x[i].reshape(Cv, T))
        in_maps.append(m)
    try:
        res = run_bass_kernel_spmd(nc, in_maps, core_ids=list(range(B)),
                                   trace=trace)
    except ModuleNotFoundError:
        # axon NTFF profiling hook unavailable in this container
        res = run_bass_kernel_spmd(nc, in_maps, core_ids=list(range(B)),
                                   trace=False)
    out = np.stack([res.results[i]["out"] for i in range(B)])
    return out.reshape(B, Cv, H, W).astype(np.float32), res


def kernel(**inputs):
    out, _ = run(**inputs)
    return out


# revision 5
# speedup vs baseline: 1.5338x; 1.5338x over previous
"""AttentionBlock (GroupNorm -> qkv -> attention -> proj -> residual) on 8 TRN2 cores.

Data-parallel over batch: B=8 samples, one per NeuronCore; no collectives.

All matmuls run in fp8e4 (e4m3) with DoubleRow perf mode: two 128-deep
K-subtiles contracted per matmul at double rate. Layouts put K-subtile
pairs adjacent in the free dim so a single 3D AP [128, 2, N] feeds each
DoubleRow operand (dual-fp8 ldweights requires a 128-wide stationary
free dim — ones vectors must be [128, 2, 128], which conveniently makes
the softmax-denominator matmul produce its result replicated across all
128 PSUM partitions, i.e. pre-broadcast):

  - h, q, k stored [128, NCT=4, T]   (partition = channel%128, dim1 = c-subtile)
  - vT stored     [128, NT=32, C]    (partition = token%128,  dim1 = s-subtile)
  - PT (exp of scores^T) [128, NT, TCH] per t-chunk, fp8
  - weights wqkvT [128, NCT, 3C], wprojT [128, NCT, C], fp8

Scores are computed transposed, ST[s,t] = sum_c k[c,s] q[c,t], so softmax's
reduction lands on partitions: exp on the ACT engine (with a constant -3
offset so all exps fit fp8 range; the offset cancels in normalization),
denominator via the DoubleRow ones-matmul above, one DVE reciprocal of the
replicated denominator gives the broadcast 1/den directly.

Schedule: the ACT engine (256 exp tiles) is the phase-3 bottleneck, so
everything else is arranged to keep its stream gapless: den+PV consume
exp'd pairs two pairs late (ACT latency off the PE in-order path), and
each chunk's last pair-consumes + normalization + proj run inside the NEXT
chunk's scores window (software pipeline across the chunk boundary).
PSUM budget (8 banks): scores 2 + den 1 + PV 4 + proj 1.

GroupNorm: bn_stats on chunked x loads (x DMAd via the SP/ACT HWDGE
queues, SBUF-resident f32 for the residual add), cross-partition group
reduce and broadcast-back via tiny exact 0/1-selection f32 matmuls.
GroupNorm's apply pass (fp8 h) is fused into the qkv chunk loop, which
interleaves q/k (ACT evac, +bias) and vT (DVE evac) production.
Weight transposes / q,k pre-scaling by C**-0.25 / v-bias folding are done
on the host in numpy - O(C^2) one-time prep.
"""

import os
import sys

for _p in ("/opt/trn_rl_repo", "/opt/pypackages"):
    if os.path.isdir(_p) and _p not in sys.path:
        sys.path.insert(0, _p)

import numpy as np
import ml_dtypes

import json as _json

import concourse.bass as bass
import concourse.tile as tile
from concourse import mybir
from concourse.bass_utils import run_bass_kernel_spmd

# Walrus's codegen (setupSyncWait) encodes at most ONE sync wait on a DMA
# instruction and errors out ("Too many sync wait commands") instead of
# splitting. Tile's scheduler freely attaches several waits. This pass hoists
# excess waits into standalone EventSemaphore instructions on the same engine
# immediately before the offending instruction — semantically identical (the
# engine's sequencer evaluates them in stream order before issuing it).
_WAIT_LIMITS = {"DMACopy": 1}
_WAIT_LIMIT_DEFAULT = 1


def _legalize_sync_waits(raw: bytes) -> bytes:
    d = _json.loads(raw)
    n_hoisted = 0
    for fn in d.get("functions", []):
        for blk in fn.get("blocks", []):
            out = []
            for inst in blk["instructions"]:
                si = inst.get("sync_info")
                waits = (si or {}).get("on_wait") or []
                limit = _WAIT_LIMITS.get(inst.get("opcode"), _WAIT_LIMIT_DEFAULT)
                if len(waits) > limit and inst.get("engine") not in (
                        None, "Unassigned"):
                    keep = waits[-limit:]
                    hoist = waits[:-limit]
                    for j, w in enumerate(hoist):
                        out.append({
                            "debug": inst.get("debug", 0),
                            "engine": inst["engine"],
                            "ins": [], "outs": [],
                            "name": f"{inst['name']}-hw{j}",
                            "opcode": "EventSemaphore",
                            "sync_info": {"on_update": [], "on_wait": [w]},
                        })
                        n_hoisted += 1
                    si["on_wait"] = keep
                out.append(inst)
            blk["instructions"] = out
    if n_hoisted:
        d.setdefault("attributes", {})
    return _json.dumps(d).encode()


def _install_wait_legalizer(nc):
    orig = nc.to_json_bytes

    def patched():
        return _legalize_sync_waits(orig())

    nc.to_json_bytes = patched

F32 = mybir.dt.float32
BF16 = mybir.dt.bfloat16
F8 = mybir.dt.float8e4
AL = mybir.AluOpType
AF = mybir.ActivationFunctionType
DR = mybir.MatmulPerfMode.DoubleRow

C = 512
G = 32          # groupnorm groups
NCT = C // 128  # 4 channel subtiles
EPS = 1e-5
TCH = 512       # t-chunk width
EXP_OFF = 3.0   # exp(s - OFF): keeps all exps < e^3 ~ 20, inside fp8 range


def build_graph(T, n_cores=8, phases=3):
    NT = T // 128
    NCH = T // TCH
    NST = TCH // 128  # s-subtiles per chunk in phase 2 (v production)
    nc = bass.Bass("TRN2", target_bir_lowering=False, debug=False,
                   num_devices=n_cores)

    x_d = nc.dram_tensor("x", [C, T], F32, kind="ExternalInput").ap()
    wqkvt_d = nc.dram_tensor("wqkvt", [128, NCT, 3 * C], F8,
                             kind="ExternalInput").ap()
    wprojt_d = nc.dram_tensor("wprojt", [128, NCT, C], F8,
                              kind="ExternalInput").ap()
    bqk_d = nc.dram_tensor("bqk", [2 * C, 1], F32, kind="ExternalInput").ap()
    bout_d = nc.dram_tensor("bout", [C, 1], F32, kind="ExternalInput").ap()
    gnw_d = nc.dram_tensor("gnw", [C, 1], F32, kind="ExternalInput").ap()
    gnb_d = nc.dram_tensor("gnb", [C, 1], F32, kind="ExternalInput").ap()
    # 0/1 selection matrices for the cross-partition GroupNorm reductions
    # (replace DRAM round trips with tiny exact f32 matmuls)
    gsel_d = nc.dram_tensor("gsel", [128, NCT, G], F32,
                            kind="ExternalInput").ap()
    gselt_d = nc.dram_tensor("gselt", [G, NCT, 128], F32,
                             kind="ExternalInput").ap()
    out_d = nc.dram_tensor("out", [C, T], F32, kind="ExternalOutput").ap()

    with tile.TileContext(nc) as tc:
        with (
            tc.tile_pool(name="singles", bufs=1) as sing,
            tc.tile_pool(name="persist", bufs=1) as pers,
        ):
            # ---- x loads FIRST, on the HWDGE queues (SP/ACT): everything
            # in phase 1 waits on these, and the Pool SWDGE queue is busy
            # with descriptor-gen for the small weight loads below ----
            x_sb = [pers.tile([128, T], F32, name=f"x{i}", tag=f"x{i}")
                    for i in range(NCT)]
            nxc = 4  # x DMA chunks per channel tile
            xq = [nc.sync, nc.scalar, nc.sync, nc.scalar]
            for ci in range(NCT):
                for j in range(nxc):
                    w = T // nxc
                    xq[ci].dma_start(
                        x_sb[ci][:, j * w:(j + 1) * w],
                        x_d[ci * 128:(ci + 1) * 128, j * w:(j + 1) * w])

            # ---- weights & constants (resident whole kernel) ----
            wqkv_sb = sing.tile([128, NCT, 3 * C], F8, name="wqkv", tag="wqkv")
            nc.gpsimd.dma_start(wqkv_sb, wqkvt_d)
            wproj_sb = sing.tile([128, NCT, C], F8, name="wproj", tag="wproj")
            nc.gpsimd.dma_start(wproj_sb, wprojt_d)
            bq_sb, bk_sb, bout_sb = [], [], []
            for i in range(NCT):
                b = sing.tile([128, 1], F32, name=f"bq{i}", tag=f"bq{i}")
                nc.gpsimd.dma_start(b, bqk_d[i * 128:(i + 1) * 128, :])
                bq_sb.append(b)
            for i in range(NCT):
                b = sing.tile([128, 1], F32, name=f"bk{i}", tag=f"bk{i}")
                nc.gpsimd.dma_start(b, bqk_d[C + i * 128:C + (i + 1) * 128, :])
                bk_sb.append(b)
            for i in range(NCT):
                b = sing.tile([128, 1], F32, name=f"bout{i}", tag=f"bout{i}")
                nc.gpsimd.dma_start(b, bout_d[i * 128:(i + 1) * 128, :])
                bout_sb.append(b)
            # dual-fp8 ldweights requires stationary free dim 128: use a
            # [128, 2, 128] ones stationary so the den matmul lands the
            # denominator REPLICATED across all 128 PSUM partitions (the
            # broadcast comes for free)
            ones8 = sing.tile([128, 2, 128], F8, name="ones8", tag="ones8")
            nc.vector.memset(ones8, 1.0)
            negoff = sing.tile([128, 1], F32, name="negoff", tag="negoff")
            nc.vector.memset(negoff, -EXP_OFF)
            # groupnorm per-channel affine coefs (computed in phase 1)
            A_sb = [sing.tile([128, 1], F32, name=f"gnA{i}", tag=f"gnA{i}")
                    for i in range(NCT)]
            B_sb = [sing.tile([128, 1], F32, name=f"gnB{i}", tag=f"gnB{i}")
                    for i in range(NCT)]

            # ---- persistent activations ----
            h_sb = pers.tile([128, NCT, T], F8, name="h", tag="h")
            q_sb = pers.tile([128, NCT, T], F8, name="q", tag="q")
            k_sb = pers.tile([128, NCT, T], F8, name="k", tag="k")
            vT_sb = pers.tile([128, NT, C], F8, name="vt", tag="vt")

            # ================= phase 1: GroupNorm stats =================
            # Cross-partition group reductions/broadcasts via tiny exact
            # 0/1-selection f32 matmuls (no DRAM round trips). x DMAs are
            # chunked so bn_stats pipelines with the loads.
            with (
                tc.tile_pool(name="gns", bufs=2) as gns,
                tc.tile_pool(name="pgn", bufs=2, space="PSUM") as pgn,
            ):
                gsel_sb = gns.tile([128, NCT, G], F32, name="gsel", tag="gsel",
                                   bufs=1)
                nc.gpsimd.dma_start(gsel_sb, gsel_d)
                gselt_sb = gns.tile([G, NCT, 128], F32, name="gselt",
                                    tag="gselt", bufs=1)
                nc.gpsimd.dma_start(gselt_sb, gselt_d)
                gnw_sb = gns.tile([128, NCT], F32, name="gnw_sb", tag="gnw",
                                  bufs=1)
                gnb_sb = gns.tile([128, NCT], F32, name="gnb_sb", tag="gnb",
                                  bufs=1)
                for ci in range(NCT):
                    nc.gpsimd.dma_start(gnw_sb[:, ci:ci + 1],
                                        gnw_d[ci * 128:(ci + 1) * 128, :])
                    nc.gpsimd.dma_start(gnb_sb[:, ci:ci + 1],
                                        gnb_d[ci * 128:(ci + 1) * 128, :])
                nbn = T // 512
                gst = pgn.tile([G, 2], F32, name="gst_ps", tag="gst")
                mvs = []
                for ci in range(NCT):
                    bns = gns.tile([128, nbn, 6], F32, name="bns", tag="bns")
                    x3 = x_sb[ci].rearrange("p (n f) -> p n f", f=512)
                    for j in range(nbn):
                        nc.vector.bn_stats(bns[:, j, :], x3[:, j, :])
                    mv = gns.tile([128, 2], F32, name="mv", tag=f"mv{ci}",
                                  bufs=1)
                    nc.vector.bn_aggr(mv, bns)
                    # mv[:,1] <- E[x^2] = mu^2 + var
                    nc.vector.scalar_tensor_tensor(
                        mv[:, 1:2], in0=mv[:, 0:1], scalar=mv[:, 0:1],
                        in1=mv[:, 1:2], op0=AL.mult, op1=AL.add)
                    mvs.append(mv)
                # group-sum the per-channel (mean, E[x^2]) rows: [32, 2]
                for ci in range(NCT):
                    nc.tensor.matmul(gst, lhsT=gsel_sb[:, ci, :], rhs=mvs[ci],
                                     start=(ci == 0), stop=(ci == NCT - 1))
                gv = gns.tile([G, 2], F32, name="gv", tag="gv")
                gsize = C // G
                nc.vector.tensor_scalar_mul(gv, gst, 1.0 / gsize)
                std = gns.tile([G, 1], F32, name="std", tag="std")
                # mu^2 - E[x^2] = -var
                nc.vector.scalar_tensor_tensor(
                    std, in0=gv[:, 0:1], scalar=gv[:, 0:1], in1=gv[:, 1:2],
                    op0=AL.mult, op1=AL.subtract)
                # var + eps
                nc.vector.tensor_scalar(std, std, -1.0, EPS,
                                        op0=AL.mult, op1=AL.add)
                nc.scalar.activation(std, std, AF.Sqrt)
                rhsb = gns.tile([G, 2], F32, name="rhsb", tag="rhsb")
                nc.vector.reciprocal(rhsb[:, 0:1], std)
                nc.vector.tensor_copy(rhsb[:, 1:2], gv[:, 0:1])
                # broadcast (rstd, mu) back to the 128-partition channel
                # layout per subtile, then A = gn_w * rstd; B = gn_b - mu * A
                for ci in range(NCT):
                    bc = pgn.tile([128, 2], F32, name="bc_ps", tag="bc")
                    nc.tensor.matmul(bc, lhsT=gselt_sb[:, ci, :], rhs=rhsb,
                                     start=True, stop=True)
                    nc.vector.tensor_mul(A_sb[ci], gnw_sb[:, ci:ci + 1],
                                         bc[:, 0:1])
                    tmp = gns.tile([128, 1], F32, name="gn_tmp", tag="tmp")
                    nc.vector.tensor_mul(tmp, bc[:, 1:2], A_sb[ci])
                    nc.vector.tensor_sub(B_sb[ci], gnb_sb[:, ci:ci + 1], tmp)

            # ========== phase 2: GN apply + qkv, fused per t-chunk ==========
            with tc.tile_pool(name="qkvp", bufs=6, space="PSUM") as qkvp:
                def h_apply(ch):
                    # h = A*x + B  (f32 -> fp8), per channel subtile
                    t0 = ch * TCH
                    for ci in range(NCT):
                        nc.vector.tensor_scalar(
                            h_sb[:, ci, t0:t0 + TCH],
                            x_sb[ci][:, t0:t0 + TCH],
                            A_sb[ci], B_sb[ci], op0=AL.mult, op1=AL.add)

                if phases >= 2:
                    h_apply(0)
                for ch in range(NCH if phases >= 2 else 0):
                    t0 = ch * TCH
                    # produce NEXT chunk's h while this chunk's matmuls run
                    # (this chunk's h was emitted one iteration ago)
                    if ch + 1 < NCH:
                        h_apply(ch + 1)

                    def qk_one(dst, dst_off, bias, ci, on_act):
                        ps = qkvp.tile([128, TCH], F32, name="qkv_ps",
                                       tag="ps")
                        for p in (0, 2):
                            nc.tensor.matmul(
                                ps,
                                lhsT=wqkv_sb[:, p:p + 2,
                                             dst_off + ci * 128:
                                             dst_off + (ci + 1) * 128],
                                rhs=h_sb[:, p:p + 2, t0:t0 + TCH],
                                start=(p == 0), stop=(p == 2),
                                perf_mode=DR)
                        if on_act:
                            nc.scalar.activation(
                                dst[:, ci, t0:t0 + TCH], ps,
                                AF.Identity, bias=bias[ci])
                        else:
                            # DVE evac (+bias) keeps the ACT engine free for
                            # the exp stream that overlaps from phase 3
                            nc.vector.tensor_scalar(
                                dst[:, ci, t0:t0 + TCH], ps, bias[ci], None,
                                op0=AL.add)

                    def v_one(j):
                        st = ch * NST + j
                        ps = qkvp.tile([128, C], F32, name="qkv_ps2",
                                       tag="ps")
                        for p in (0, 2):
                            nc.tensor.matmul(
                                ps,
                                lhsT=h_sb[:, p:p + 2,
                                          st * 128:(st + 1) * 128],
                                rhs=wqkv_sb[:, p:p + 2, 2 * C:3 * C],
                                start=(p == 0), stop=(p == 2),
                                perf_mode=DR)
                        nc.vector.tensor_copy(vT_sb[:, st, :], ps)

                    # k,v only: q is deferred into phase 3 (produced one
                    # chunk ahead there, evacuated by DVE) so phase 2
                    # delivers k/vT sooner and the ACT engine sheds the q
                    # evacuations that would otherwise compete with the exp
                    # stream
                    for ci in range(NCT):
                        qk_one(k_sb, C, bk_sb, ci, True)
                        v_one(ci)

            # ================= phase 3: attention + proj =================
            with (
                tc.tile_pool(name="p3s", bufs=2) as p3s,
                tc.tile_pool(name="p3w", bufs=2) as p3w,
                tc.tile_pool(name="psp", bufs=2, space="PSUM") as psp,
                tc.tile_pool(name="pden", bufs=1, space="PSUM") as pden,
                tc.tile_pool(name="ppv", bufs=1, space="PSUM") as ppv,
                tc.tile_pool(name="ppr", bufs=1, space="PSUM") as ppr,
            ):
                def emit_proj(t0, at):
                    # proj + bias + residual (software-pipelined one chunk
                    # behind the scores/PV of the next chunk)
                    for oi in range(NCT):
                        pr = ppr.tile([128, TCH], F32, name="pr_ps", tag="pr")
                        for p in (0, 2):
                            nc.tensor.matmul(
                                pr,
                                lhsT=wproj_sb[:, p:p + 2,
                                              oi * 128:(oi + 1) * 128],
                                rhs=at[:, p:p + 2, :],
                                start=(p == 0), stop=(p == 2),
                                perf_mode=DR)
                        osb = p3w.tile([128, TCH], F32, name="osb", tag="osb",
                                       bufs=3)
                        nc.vector.scalar_tensor_tensor(
                            osb, in0=pr, scalar=bout_sb[oi],
                            in1=x_sb[oi][:, t0:t0 + TCH],
                            op0=AL.add, op1=AL.add)
                        nc.sync.dma_start(
                            out_d[oi * 128:(oi + 1) * 128, t0:t0 + TCH], osb)

                # Software pipeline: each chunk's last two pair-consumes,
                # normalization, and proj run inside the NEXT chunk's
                # scores window so neither the PE nor the ACT exp stream
                # ever pauses at a chunk boundary.
                def make_chunk_ops(t0, pt, dps, pvs):
                    state = {}

                    def consume_pair(i):
                        nc.tensor.matmul(
                            dps, lhsT=ones8,
                            rhs=pt[:, 2 * i:2 * i + 2, :],
                            start=(i == 0), stop=(i == NT // 2 - 1),
                            perf_mode=DR)
                        for ci in range(NCT):
                            nc.tensor.matmul(
                                pvs[ci],
                                lhsT=vT_sb[:, 2 * i:2 * i + 2,
                                           ci * 128:(ci + 1) * 128],
                                rhs=pt[:, 2 * i:2 * i + 2, :],
                                start=(i == 0), stop=(i == NT // 2 - 1),
                                perf_mode=DR)

                    def finalize():
                        # dps holds den replicated on every partition: one
                        # DVE reciprocal gives the broadcast 1/den in SBUF
                        bcs = p3w.tile([128, TCH], F32, name="bc_sb",
                                       tag="bc_sb")
                        nc.vector.reciprocal(bcs, dps)
                        at = p3w.tile([128, NCT, TCH], F8, name="at",
                                      tag="at")
                        for ci in range(NCT):
                            nc.vector.tensor_mul(at[:, ci, :], pvs[ci], bcs)
                        state["at"] = at

                    def proj():
                        emit_proj(t0, state["at"])

                    return consume_pair, finalize, proj

                def q_one(ch, ci):
                    # q for chunk ch: rides the proj PSUM bank (idle
                    # mid-window), DVE evacuates (+bias) into fp8
                    t0q = ch * TCH
                    ps = ppr.tile([128, TCH], F32, name="q_ps", tag="pr")
                    for p in (0, 2):
                        nc.tensor.matmul(
                            ps,
                            lhsT=wqkv_sb[:, p:p + 2, ci * 128:(ci + 1) * 128],
                            rhs=h_sb[:, p:p + 2, t0q:t0q + TCH],
                            start=(p == 0), stop=(p == 2), perf_mode=DR)
                    nc.vector.tensor_scalar(
                        q_sb[:, ci, t0q:t0q + TCH], ps, bq_sb[ci], None,
                        op0=AL.add)

                if phases >= 3:
                    for ci in range(NCT):
                        q_one(0, ci)
                prev = None  # (consume_pair, finalize, proj) awaiting tail
                for tci in range(NCH if phases >= 3 else 0):
                    t0 = tci * TCH
                    # scores^T + exp -> PT tiles [s, t] fp8; den + PV
                    # accumulate interleaved, two exp'd pairs behind, so
                    # ACT exp latency stays off the PE critical path
                    pt = p3s.tile([128, NT, TCH], F8, name="pt", tag="pt")
                    dps = pden.tile([128, TCH], F32, name="den_ps", tag="den")
                    pvs = [ppv.tile([128, TCH], F32, name=f"pv{ci}",
                                    tag=f"pv{ci}") for ci in range(NCT)]
                    ops = make_chunk_ops(t0, pt, dps, pvs)
                    consume_pair = ops[0]
                    # lag-2 pair consumption keeps ACT latency off the PE
                    # in-order path
                    lag = 2
                    for st in range(NT):
                        sp = psp.tile([128, TCH], F32, name="st_ps", tag="st")
                        for p in (0, 2):
                            nc.tensor.matmul(
                                sp,
                                lhsT=k_sb[:, p:p + 2,
                                          st * 128:(st + 1) * 128],
                                rhs=q_sb[:, p:p + 2, t0:t0 + TCH],
                                start=(p == 0), stop=(p == 2),
                                perf_mode=DR)
                        nc.scalar.activation(pt[:, st, :], sp, AF.Exp,
                                             bias=negoff)
                        if st % 2 == 1 and st >= 2 * lag + 1:
                            consume_pair(st // 2 - lag)
                        if prev is not None:
                            if st == 1:
                                prev[0](NT // 2 - 2)
                            elif st == 3:
                                prev[0](NT // 2 - 1)
                                prev[1]()  # recip + at-mults
                            elif st == 5:
                                prev[2]()  # proj + epilogue
                                prev = None
                        if st in (15, 17, 19, 21) and tci + 1 < NCH:
                            q_one(tci + 1, (st - 15) // 2)
                    prev = ops
                if prev is not None:
                    prev[0](NT // 2 - 2)
                    prev[0](NT // 2 - 1)
                    prev[1]()
                    prev[2]()
    _install_wait_legalizer(nc)
    return nc


def host_prep(gn_w, gn_b, qkv_w, qkv_b, proj_w, proj_b):
    """One-time O(C^2) weight prep in numpy -> per-core replicated inputs."""
    scale = float(C) ** -0.25
    e4 = ml_dtypes.float8_e4m3
    wq, wk, wv = qkv_w[:C], qkv_w[C:2 * C], qkv_w[2 * C:]
    wt = np.concatenate([wq.T * scale, wk.T * scale, wv.T], axis=1)  # [C, 3C]
    wqkvt = np.ascontiguousarray(
        wt.reshape(NCT, 128, 3 * C).transpose(1, 0, 2)).astype(e4)
    wpt = np.ascontiguousarray(proj_w.T)  # [C, C]
    wprojt = np.ascontiguousarray(
        wpt.reshape(NCT, 128, C).transpose(1, 0, 2)).astype(e4)
    bqk = (qkv_b[:2 * C] * scale).astype(np.float32).reshape(2 * C, 1)
    bout = (proj_w @ qkv_b[2 * C:] + proj_b).astype(np.float32).reshape(C, 1)
    # selection matrices: channel c = ci*128 + p belongs to group c//16
    gsize = C // G
    gsel = np.zeros((128, NCT, G), np.float32)
    gselt = np.zeros((G, NCT, 128), np.float32)
    for ci in range(NCT):
        for p in range(128):
            g = (ci * 128 + p) // gsize
            gsel[p, ci, g] = 1.0
            gselt[g, ci, p] = 1.0
    return {
        "wqkvt": wqkvt, "wprojt": wprojt, "bqk": bqk, "bout": bout,
        "gnw": gn_w.astype(np.float32).reshape(C, 1),
        "gnb": gn_b.astype(np.float32).reshape(C, 1),
        "gsel": gsel, "gselt": gselt,
    }


_graph_cache = {}


def get_graph(T, B):
    key = (T, B)
    if key not in _graph_cache:
        _graph_cache[key] = build_graph(T, n_cores=B)
    return _graph_cache[key]


def run(x, gn_w, gn_b, qkv_w, qkv_b, proj_w, proj_b, trace=False):
    x = np.asarray(x, np.float32)
    B, Cv, H, W = x.shape
    T = H * W
    shared = host_prep(np.asarray(gn_w), np.asarray(gn_b),
                       np.asarray(qkv_w), np.asarray(qkv_b),
                       np.asarray(proj_w), np.asarray(proj_b))
    nc = get_graph(T, B)
    in_maps = []
    for i in range(B):
        m = dict(shared)
        m["x"] = np.ascontiguousarray(x[i].reshape(Cv, T))
        in_maps.append(m)
    try:
        res = run_bass_kernel_spmd(nc, in_maps, core_ids=list(range(B)),
                                   trace=trace)
    except ModuleNotFoundError:
        # axon NTFF profiling hook unavailable in this container
        res = run_bass_kernel_spmd(nc, in_maps, core_ids=list(range(B)),
                                   trace=False)
    out = np.stack([res.results[i]["out"] for i in range(B)])
    return out.reshape(B, Cv, H, W).astype(np.float32), res


def kernel(**inputs):
    out, _ = run(**inputs)
    return out
